# revision 49
# baseline (speedup 1.0000x reference)
"""BPR pairwise softplus loss on 8 Trainium2 NeuronCores.

loss = mean_b sum_{i<K, j>=K, both valid} softplus(pred[b,j] - pred[b,i])

Strategy (data parallel over batch, 32 rows/core), folding FOUR negatives
per ln via elementary symmetric polynomials:

  prod_{m=1..4} (1 + F*E_m) = 1 + F*c1 + F^2*c2 + F^3*c3 + F^4*c4
  =>  sum_m softplus(n_m - p) = ln(1 + sum_k F^k c_k),  F = exp(-p), E = exp(n)

The per-(pos, quad) evaluation is ONE matmul per row pair with contraction
dim 8 = (power k, row-of-pair b): stationary lhsT[(k,b), p] holds
interleave-masked F^k, moving rhs[(k,b), (u, j)] holds quad coefficients.
The matmul charges only output free size, so packing the powers into the
contraction dim quarters PE time vs. accumulation chains.

Invalid slots (target == -1) are folded into the prediction on the host
during sharding: invalid positives -> +50 (F = e^-50 -> 0), invalid
negatives -> -50 (E = e^-50 -> 0), so the device needs no target tensor,
no masking ops and no mask DMA.

Compute engines require all operands to start on the same partition, and a
DMA requires each side's partition coords to be a nested prefix of the
iteration order.  Both are satisfied by REPLICATING pred 4x across
partition blocks in the input DMA itself (stride-0 DRAM reads): partition
= 32*k + row.  Per-power ops then run on same-base block slices, and the
operand packs become plain tile-to-tile DMAs (flat-order reshapes).

  - ScalarE: exp over negatives (all replicas in one op), two F exps, two
    Ln(x+1) passes over PSUM with accum_out row sums.
  - VectorE: pair fold, per-block quad coeffs, interleave-mask selects.
  - Pool/GpSimd: F power chain, lhsT SWDGE pack.
  - per-partition partials DMA'd out; host sums 8x128xNPASS / B.
"""
import sys

sys.path.insert(0, "/opt/trn_rl_repo")

import numpy as np
import ml_dtypes

import concourse.bass as bass
import concourse.mybir as mybir
from concourse import bacc
import concourse.hw_specs as hw_specs
from concourse.tile import TileContext
from concourse.bass_utils import run_bass_kernel_spmd

B, N, K = 256, 512, 64
NC = 8
RPC = B // NC            # 32 batch rows per core
NPAIR = RPC // 2         # 16 row pairs (u paired with u+16)
NEG = N - K              # 448 negatives per row
G = 4                    # negatives folded per ln
NQ = NEG // G            # 112 quad groups per row
NPASS = 2                # Ln passes (2 PSUM banks each)
N_WARM = 65              # PE clock warm-up dummy matmuls
PRED_SPLIT = True        # split pred DMA into negs + pos
P_INPLACE = False       # build F powers by in-place block mults
POOLS4 = False           # four tile pools vs two

_PROG_CACHE = {}

EXP = mybir.ActivationFunctionType.Exp
LN = mybir.ActivationFunctionType.Ln
F32 = mybir.dt.float32
BF16 = mybir.dt.bfloat16


def _patch_act_tables():
    """Make natural_log_exp_and_others the only table set advertising exp/ln
    so Bacc's table-load pass emits a single ACT_TABLE_LOAD."""
    if getattr(hw_specs.get_activation_tables, "_bpr_patched", False):
        return
    orig_fn = hw_specs.get_activation_tables

    def patched(arch):
        d = orig_fn(arch)
        out = {}
        for name, funcs in d.items():
            if name != "natural_log_exp_and_others" and (EXP in funcs
                                                         or LN in funcs):
                funcs = funcs - {EXP, LN}
            out[name] = funcs
        return out

    patched._bpr_patched = True
    hw_specs.get_activation_tables = patched
    bacc.get_activation_tables = patched


def build_program(nreps: int = 1):
    if nreps in _PROG_CACHE:
        return _PROG_CACHE[nreps]
    _patch_act_tables()
    nc = bacc.Bacc("TRN2", target_bir_lowering=False, debug=False,
                   num_devices=NC)
    pred = nc.dram_tensor("pred", [RPC, N], BF16, kind="ExternalInput")
    # interleave mask, replicated per power block: rows 32k+(0..15) keep
    # cols 0:64, rows 32k+(16..31) keep cols 64:128
    pm = nc.dram_tensor("pm", [4 * RPC, 2 * K], BF16, kind="ExternalInput")
    y = nc.dram_tensor("y", [nreps, 128, NPASS], F32, kind="ExternalOutput")

    mul = mybir.AluOpType.mult
    add = mybir.AluOpType.add

    from contextlib import ExitStack
    with TileContext(nc) as tc, ExitStack() as st:
        io = st.enter_context(tc.tile_pool(name="io", bufs=1))
        ps = st.enter_context(tc.tile_pool(name="ps", bufs=1, space="PSUM"))
        if POOLS4:
            mmp = st.enter_context(tc.tile_pool(name="mm", bufs=2))
            scr = st.enter_context(tc.tile_pool(name="scr", bufs=2))
        else:
            mmp = io
            scr = io
        if True:
            # Trigger the exp/ln activation-table load ASAP (~1.3us on
            # ScalarE, overlapping the input DMA).
            d0 = io.tile([128, 1], F32, tag="d0")
            nc.vector.memset(d0, 0.0)
            d1 = io.tile([128, 1], BF16, tag="d1")
            nc.scalar.activation(d1, d0, EXP)

            for rep in range(nreps):
                # negatives first on the fast SP queue (gates the exp
                # chain), positives separately
                pred_sb = io.tile([4 * RPC, N], BF16, tag="pred")
                if PRED_SPLIT:
                    nc.sync.dma_start(
                        out=pred_sb[:, K:N],
                        in_=pred[:, K:N].unsqueeze(0).broadcast_to(
                            [4, RPC, NEG]))
                    nc.sync.dma_start(
                        out=pred_sb[:, 0:K],
                        in_=pred[:, 0:K].unsqueeze(0).broadcast_to(
                            [4, RPC, K]))
                else:
                    nc.sync.dma_start(
                        out=pred_sb,
                        in_=pred[:].unsqueeze(0).broadcast_to([4, RPC, N]))
                pm_sb = io.tile([4 * RPC, 2 * K], BF16, tag="pm")
                nc.gpsimd.dma_start(out=pm_sb, in_=pm[:])

                # ---- E side (critical): E = exp(pred_neg), two halves;
                # the F exps run between them so the F-power chain starts
                # early ----
                HN = NEG // 2
                e_raw = io.tile([4 * RPC, NEG], BF16, tag="eraw")
                nc.scalar.activation(e_raw[:, 0:HN], pred_sb[:, K:K + HN],
                                     EXP)

                # F exp once; the duplicate half is a Pool copy
                fdup = io.tile([4 * RPC, 2 * K], BF16, tag="fdup")
                nc.scalar.activation(fdup[:, 0:K], pred_sb[:, 0:K], EXP,
                                     scale=-1.0)

                nc.scalar.activation(e_raw[:, HN:NEG], pred_sb[:, K + HN:N],
                                     EXP)
                nc.gpsimd.tensor_copy(fdup[:, K:2 * K], fdup[:, 0:K])

                # F^2 (and F^3 for the select variant) on Pool
                t2 = io.tile([4 * RPC, 2 * K], BF16, tag="t2")
                nc.gpsimd.tensor_tensor(t2, fdup, fdup, mul)
                if not P_INPLACE:
                    t3 = io.tile([4 * RPC, 2 * K], BF16, tag="t3")
                    nc.gpsimd.tensor_tensor(t3, t2, fdup, mul)

                # pair fold per half: pairs (x, x+112) within each half;
                # am = [aA | aB | mA | mB]; adds on DVE, mults on Pool so
                # the post-E-B DVE stream (which gates the rhs pack) is
                # shorter
                am_a = io.tile([4 * RPC, 2 * NQ], BF16, tag="ama")
                am_m = io.tile([4 * RPC, 2 * NQ], BF16, tag="amm")
                nc.vector.tensor_tensor(am_a[:, 0:NQ], e_raw[:, 0:NQ],
                                        e_raw[:, NQ:2 * NQ], add)
                nc.vector.tensor_tensor(am_m[:, 0:NQ], e_raw[:, 0:NQ],
                                        e_raw[:, NQ:2 * NQ], mul)
                nc.vector.tensor_tensor(am_a[:, NQ:2 * NQ],
                                        e_raw[:, 2 * NQ:3 * NQ],
                                        e_raw[:, 3 * NQ:4 * NQ], add)
                nc.vector.tensor_tensor(am_m[:, NQ:2 * NQ],
                                        e_raw[:, 2 * NQ:3 * NQ],
                                        e_raw[:, 3 * NQ:4 * NQ], mul)

                # quad coeffs per power block: c1 = a1+a2,
                # c2 = m1+m2+a1*a2, c3 = a1*m2+a2*m1, c4 = m1*m2
                # (quads {x, x+112, x+224, x+336})
                q = io.tile([4 * RPC, NQ], BF16, tag="q")
                t1 = scr.tile([4 * RPC, NQ], BF16, tag="t1")
                R1, R2, R3, R4 = RPC, 2 * RPC, 3 * RPC, 4 * RPC
                A1 = slice(0, NQ)
                A2 = slice(NQ, 2 * NQ)
                # am_m (Pool) lands later than am_a: mult-free ops first
                nc.vector.tensor_tensor(q[0:R1], am_a[0:R1, A1],
                                        am_a[0:R1, A2], add)
                nc.vector.tensor_tensor(t1[R1:R2], am_a[R1:R2, A1],
                                        am_a[R1:R2, A2], mul)
                nc.vector.tensor_tensor(t1[R2:R3], am_a[R2:R3, A1],
                                        am_m[R2:R3, A2], mul)
                nc.vector.tensor_tensor(q[R2:R3], am_a[R2:R3, A2],
                                        am_m[R2:R3, A1], mul)
                nc.vector.tensor_tensor(q[R1:R2], am_m[R1:R2, A1],
                                        am_m[R1:R2, A2], add)
                nc.vector.tensor_tensor(q[R1:R2], q[R1:R2], t1[R1:R2], add)
                nc.vector.tensor_tensor(q[R2:R3], q[R2:R3], t1[R2:R3], add)
                nc.vector.tensor_tensor(q[R3:R4], am_m[R3:R4, A1],
                                        am_m[R3:R4, A2], mul)

                # pack quad coeffs: rhs[(k,b), (u,j)] - flat reshape, on the
                # fast SP HWDGE queue, issued as soon as the quads land
                rhs = mmp.tile([2 * G, NQ * NPAIR], BF16, tag="rhs")
                nc.sync.dma_start(out=rhs, in_=q)

                # interleave-masked F powers
                P = io.tile([4 * RPC, 2 * K], BF16, tag="P")
                if P_INPLACE:
                    # pm is 0/1 so pm^2 = pm: P = F*pm everywhere, then
                    # block k *= F^(k-1) factors in place
                    nc.vector.tensor_tensor(P, fdup, pm_sb, mul)
                    nc.vector.tensor_tensor(P[R1:R2], P[R1:R2],
                                            fdup[R1:R2], mul)
                    nc.vector.tensor_tensor(P[R2:R3], P[R2:R3], t2[R2:R3],
                                            mul)
                    nc.vector.tensor_tensor(P[R3:R4], P[R3:R4],
                                            fdup[R3:R4], mul)
                    nc.vector.tensor_tensor(P[R3:R4], P[R3:R4], t2[R3:R4],
                                            mul)
                else:
                    t4 = io.tile([4 * RPC, 2 * K], BF16, tag="t4")
                    nc.vector.tensor_tensor(t4, t2, t2, mul)
                    nc.vector.tensor_tensor(P[0:R1], fdup[0:R1],
                                            pm_sb[0:R1], mul)
                    nc.vector.tensor_tensor(P[R1:R2], t2[R1:R2],
                                            pm_sb[R1:R2], mul)
                    nc.vector.tensor_tensor(P[R3:R4], t4[R3:R4],
                                            pm_sb[R3:R4], mul)
                    nc.vector.tensor_tensor(P[R2:R3], t3[R2:R3],
                                            pm_sb[R2:R3], mul)

                # pack F powers: lhsT[(k,b), (u,p)] - flat reshape, second
                # SP HWDGE transfer (pipelines behind rhs)
                lhsT = mmp.tile([2 * G, 2 * K * NPAIR], BF16, tag="lhsT")
                nc.sync.dma_start(out=lhsT, in_=P)

                # one matmul per rowpair u: psum[p, j] = sum_k F^k c_k
                # slot(u) = 512*(u//4) + 112*(u%4); 2 banks per Ln pass,
                # separate tiles so each Ln pass depends only on its half
                pt0 = ps.tile([128, 2 * 512], F32, tag="ps0")
                pt1 = ps.tile([128, 2 * 512], F32, tag="ps1")
                pts = [pt0, pt1]

                # warm matmuls gated on the early pm DMA keep the PE clock
                # ramping from ~3us so the real stream runs at full speed
                # (they write pad columns of the psum tiles)
                for w in range(N_WARM):
                    nc.tensor.matmul(pt0[:, 448:512], pm_sb[0:2, 0:2 * K],
                                     pm_sb[0:2, 0:K], start=True, stop=True)
                for u in range(NPAIR):
                    pt = pts[u // 8]
                    uu = u % 8
                    out_sl = pt[:, 512 * (uu // 4) + NQ * (uu % 4):
                                512 * (uu // 4) + NQ * (uu % 4) + NQ]
                    nc.tensor.matmul(out_sl,
                                     lhsT[:, 2 * K * u: 2 * K * (u + 1)],
                                     rhs[:, NQ * u: NQ * (u + 1)],
                                     start=True, stop=True)

                # ln(1 + psum), accumulated per partition; NPASS passes of
                # 2 banks each
                partials = mmp.tile([128, NPASS], F32, tag="part")
                sout = scr.tile([128, 2 * 4 * NQ], BF16, tag="scr")
                for i in range(NPASS):
                    nc.scalar.activation(
                        sout.rearrange("p (b x) -> p b x", x=4 * NQ),
                        pts[i].rearrange("p (b x) -> p b x",
                                         x=512)[:, :, 0:4 * NQ],
                        LN, bias=1.0,
                        accum_out=partials[:, i:i + 1])

                nc.sync.dma_start(out=y[rep], in_=partials)

    nc.finalize()
    _PROG_CACHE[nreps] = (nc, ())
    return nc, ()


def _pm_const():
    pmv = np.zeros((4 * RPC, 2 * K), dtype=ml_dtypes.bfloat16)
    for k in range(4):
        pmv[32 * k:32 * k + NPAIR, 0:K] = 1
        pmv[32 * k + NPAIR:32 * k + RPC, K:2 * K] = 1
    return pmv


def make_in_maps(prediction, target, consts):
    # fold validity into the prediction: invalid positives -> +50
    # (F = e^-50 -> 0), invalid negatives -> -50 (E = e^-50 -> 0)
    fill = np.empty((1, N), np.float32)
    fill[:, 0:K] = 50.0
    fill[:, K:N] = -50.0
    pred_m = np.where(target == -1, fill,
                      prediction).astype(ml_dtypes.bfloat16)
    pmv = _pm_const()
    in_maps = []
    for c in range(NC):
        in_maps.append({
            "pred": np.ascontiguousarray(pred_m[c * RPC:(c + 1) * RPC]),
            "pm": pmv,
        })
    return in_maps


def kernel(prediction, target):
    nc, consts = build_program(1)
    in_maps = make_in_maps(prediction, target, consts)
    res = run_bass_kernel_spmd(nc, in_maps, core_ids=list(range(NC)))
    total = sum(float(res.results[c]["y"][0].sum(dtype=np.float64))
                for c in range(NC))
    return np.float32(total / B)
